# revision 2
# baseline (speedup 1.0000x reference)
"""Trainium2 Bass kernel for nn_DenseEdgeModel (gnn_message_passing).

Reference computation (all 1x1 convs == per-pixel matmuls over channels):
    h    = MLP3(x)                    # 3x (c->c) with ReLU between    [B,C,H,W]
    flat = h as [B*H*W, C]
    xp   = flat[primary_idx]          # [B,PK,C] -> [B,C,PK]
    xc   = flat[compare_idx]          # [B,CK,C] -> [B,C,CK]
    xx   = (xp[..,:,None]-xc[..,None,:])**2          # [B,C,PK,CK]
    g    = relu(W1@xx+b1); g = relu(W2@g+b2)         # over C
    out  = W3@g + b3                  # [B,2,PK,CK]

Sharding (8 cores): data-parallel over batch (4 cores per batch), and the
PK axis split 4-ways within each batch -> each core owns 64 primary
indices of one batch and all 256 compare indices of that batch.

Because the pre-MLP is per-pixel, gather commutes with it:
MLP(x)[idx] == MLP(x[idx]). The host therefore only *slices* (gathers
rows of x for each core's indices and transposes to channel-major) --
zero FLOPs on host; every matmul/activation runs on device.

Device kernel per core (all fp32 data, float32r matmul streaming):
  stage 1: pre-MLP on the 320 gathered pixel columns  [c,320]
  stage 2: for each pair of primary columns p: build xx tiles
           [c,2*256] = (F[:,p] - F[:,64:320])^2 via ScalarE
           Square(scale=-1,bias), then the 2-layer post-MLP as PE
           matmuls (K=256 split in 2 chunks, fused bias+relu on
           VectorE), final (c->2) matmul + bias, DMA out.
"""

import numpy as np

import concourse.bass as bass
import concourse.tile as tile
from concourse import bacc, mybir
from concourse.bass_utils import run_bass_kernel_spmd

# Problem constants (hardcoded per harness contract)
B, C, H, W = 2, 256, 32, 32
PK, CK = 256, 256
N_CORES = 8
CORES_PER_BATCH = N_CORES // B          # 4
P_SHARD = PK // CORES_PER_BATCH         # 64 primary indices per core
NJ = P_SHARD + CK                       # 320 gathered pixel columns per core
PAIR = 2                                # primary columns per stage-2 group
NF = PAIR * CK                          # 512 = stage-2 matmul free dim
F32 = mybir.dt.float32
MM_DT = mybir.dt.float32r               # full-rate 4-byte matmul streaming
AF = mybir.ActivationFunctionType
OP = mybir.AluOpType


def _build_nc():
    nc = bacc.Bacc("TRN2", target_bir_lowering=False, debug=False)

    xgT = nc.dram_tensor("xgT", [C, NJ], F32, kind="ExternalInput")
    pre_wT = nc.dram_tensor("pre_wT", [3, C, C], F32, kind="ExternalInput")
    pre_b = nc.dram_tensor("pre_b", [3, C], F32, kind="ExternalInput")
    post_wT = nc.dram_tensor("post_wT", [2, C, C], F32, kind="ExternalInput")
    post_b = nc.dram_tensor("post_b", [2, C], F32, kind="ExternalInput")
    w3T = nc.dram_tensor("w3T", [C, 2], F32, kind="ExternalInput")
    b3 = nc.dram_tensor("b3", [2], F32, kind="ExternalInput")
    out = nc.dram_tensor("out", [2, P_SHARD, CK], F32, kind="ExternalOutput")

    with tile.TileContext(nc) as tc:
        _kernel_body(nc, tc, xgT, pre_wT, pre_b, post_wT, post_b, w3T, b3, out)
    nc.compile()
    return nc


def _kernel_body(nc, tc, xgT, pre_wT, pre_b, post_wT, post_b, w3T, b3, out):
    with (
        tc.tile_pool(name="consts", bufs=1) as consts,
        tc.tile_pool(name="feats", bufs=1) as feats,
        tc.tile_pool(name="work", bufs=2) as work,
        tc.tile_pool(name="psum", bufs=2, space="PSUM") as psum,
    ):
        # ---- weights / biases to SBUF (replicated, loaded once) ----
        def col(ap_1d):  # [n] -> [n,1]
            return ap_1d.rearrange("(n one) -> n one", one=1)

        wpre = [[consts.tile([128, C], MM_DT, name=f"wpre_{i}_{cc}")
                 for cc in range(2)] for i in range(3)]
        bpre = [[consts.tile([128, 1], F32, name=f"bpre_{i}_{oc}")
                 for oc in range(2)] for i in range(3)]
        for i in range(3):
            for cc in range(2):
                nc.sync.dma_start(out=wpre[i][cc][:],
                                  in_=pre_wT[i, cc * 128:(cc + 1) * 128, :].bitcast(MM_DT))
            for oc in range(2):
                nc.sync.dma_start(out=bpre[i][oc][:],
                                  in_=col(pre_b[i, oc * 128:(oc + 1) * 128]))
        wpost = [[consts.tile([128, C], MM_DT, name=f"wpost_{l}_{cc}")
                  for cc in range(2)] for l in range(2)]
        bpost = [[consts.tile([128, 1], F32, name=f"bpost_{l}_{oc}")
                  for oc in range(2)] for l in range(2)]
        for l in range(2):
            for cc in range(2):
                nc.sync.dma_start(out=wpost[l][cc][:],
                                  in_=post_wT[l, cc * 128:(cc + 1) * 128, :].bitcast(MM_DT))
            for oc in range(2):
                nc.sync.dma_start(out=bpost[l][oc][:],
                                  in_=col(post_b[l, oc * 128:(oc + 1) * 128]))
        w3 = [consts.tile([128, 2], MM_DT, name=f"w3_{cc}") for cc in range(2)]
        for cc in range(2):
            nc.sync.dma_start(out=w3[cc][:],
                              in_=w3T[cc * 128:(cc + 1) * 128, :].bitcast(MM_DT))
        b3t = consts.tile([2, 1], F32, name="b3t")
        nc.sync.dma_start(out=b3t[:], in_=col(b3.ap()))

        # ---- stage 1: pre-MLP over the 320 gathered pixel columns ----
        cur = [feats.tile([128, NJ], MM_DT, name=f"xg_{cc}") for cc in range(2)]
        for cc in range(2):
            nc.sync.dma_start(out=cur[cc][:],
                              in_=xgT[cc * 128:(cc + 1) * 128, :].bitcast(MM_DT))
        for i in range(3):
            nxt = [feats.tile([128, NJ], MM_DT if i < 2 else F32,
                              name=f"feat{i}_{oc}") for oc in range(2)]
            for oc in range(2):
                ps = psum.tile([128, NJ], F32, name=f"ps_s1_{i}_{oc}",
                               tag="ps_s1")
                nc.tensor.matmul(
                    out=ps[:],
                    lhsT=wpre[i][0][:, oc * 128:(oc + 1) * 128],
                    rhs=cur[0][:], start=True, stop=False)
                nc.tensor.matmul(
                    out=ps[:],
                    lhsT=wpre[i][1][:, oc * 128:(oc + 1) * 128],
                    rhs=cur[1][:], start=False, stop=True)
                nc.scalar.activation(
                    out=nxt[oc][:], in_=ps[:],
                    func=AF.Relu if i < 2 else AF.Identity,
                    bias=bpre[i][oc][:, 0:1], scale=1.0)
            cur = nxt
        F = cur  # [128, NJ] x 2 chunks; cols 0:P_SHARD primary, P_SHARD: compare

        # ---- stage 2: pairwise + post-MLP, PAIR primary columns per group ----
        for g in range(P_SHARD // PAIR):
            xx = [work.tile([128, NF], MM_DT, name=f"xx_{cc}", tag=f"xx_{cc}")
                  for cc in range(2)]
            for s in range(PAIR):
                p = g * PAIR + s
                for cc in range(2):
                    # xx = (xp - xc)^2 = Square(-1*xc + xp)
                    nc.scalar.activation(
                        out=xx[cc][:, s * CK:(s + 1) * CK],
                        in_=F[cc][:, P_SHARD:NJ],
                        func=AF.Square,
                        bias=F[cc][:, p:p + 1], scale=-1.0)
            hcur = xx
            for l in range(2):
                hnxt = [work.tile([128, NF], MM_DT, name=f"h{l}_{oc}",
                                  tag=f"h{l}_{oc}") for oc in range(2)]
                for oc in range(2):
                    ps = psum.tile([128, NF], F32, name=f"ps_l{l}_{oc}",
                                   tag=f"ps_l{l}")
                    nc.tensor.matmul(
                        out=ps[:],
                        lhsT=wpost[l][0][:, oc * 128:(oc + 1) * 128],
                        rhs=hcur[0][:], start=True, stop=False)
                    nc.tensor.matmul(
                        out=ps[:],
                        lhsT=wpost[l][1][:, oc * 128:(oc + 1) * 128],
                        rhs=hcur[1][:], start=False, stop=True)
                    # fused bias-add + relu on VectorE (reads PSUM)
                    nc.vector.tensor_scalar(
                        out=hnxt[oc][:], in0=ps[:],
                        scalar1=bpost[l][oc][:, 0:1], scalar2=0.0,
                        op0=OP.add, op1=OP.max)
                hcur = hnxt
            ps3 = psum.tile([2, NF], F32, name="ps3", tag="ps3")
            nc.tensor.matmul(out=ps3[:], lhsT=w3[0][:],
                             rhs=hcur[0][:], start=True, stop=False)
            nc.tensor.matmul(out=ps3[:], lhsT=w3[1][:],
                             rhs=hcur[1][:], start=False, stop=True)
            ob = work.tile([2, NF], F32, name="ob", tag="ob", bufs=4)
            nc.vector.tensor_scalar(out=ob[:], in0=ps3[:],
                                    scalar1=b3t[:, 0:1], scalar2=None,
                                    op0=OP.add)
            nc.sync.dma_start(
                out=out[:, g * PAIR:(g + 1) * PAIR, :],
                in_=ob.rearrange("j (s q) -> j s q", s=PAIR))


_NC_CACHE = {}


def _get_nc():
    if "nc" not in _NC_CACHE:
        _NC_CACHE["nc"] = _build_nc()
    return _NC_CACHE["nc"]


def _shard_inputs(x, primary_indices, compare_indices, pre_w, pre_b,
                  post_w, post_b, post_out_w, post_out_b):
    """Host-side sharding: per-core index slicing + row gather of x (pure
    data movement -- the pre-MLP commutes with the gather) and weight
    transposes. Returns the 8 per-core input maps."""
    x = np.asarray(x, dtype=np.float32)
    x_rows = np.ascontiguousarray(x.transpose(0, 2, 3, 1)).reshape(B * H * W, C)
    pre_wT = np.ascontiguousarray(
        np.asarray(pre_w, dtype=np.float32).transpose(0, 2, 1))
    post_wT = np.ascontiguousarray(
        np.asarray(post_w, dtype=np.float32).transpose(0, 2, 1))
    w3T = np.ascontiguousarray(np.asarray(post_out_w, dtype=np.float32).T)
    pre_b = np.ascontiguousarray(np.asarray(pre_b, dtype=np.float32))
    post_b = np.ascontiguousarray(np.asarray(post_b, dtype=np.float32))
    b3 = np.ascontiguousarray(np.asarray(post_out_b, dtype=np.float32))
    primary_indices = np.asarray(primary_indices)
    compare_indices = np.asarray(compare_indices)

    in_maps = []
    for core in range(N_CORES):
        b = core // CORES_PER_BATCH
        ps = (core % CORES_PER_BATCH) * P_SHARD
        rows = np.concatenate([
            primary_indices[b, ps:ps + P_SHARD].astype(np.int64),
            compare_indices[b].astype(np.int64),
        ])
        xg_T = np.ascontiguousarray(x_rows[rows].T)  # [C, NJ]
        in_maps.append({
            "xgT": xg_T,
            "pre_wT": pre_wT,
            "pre_b": pre_b,
            "post_wT": post_wT,
            "post_b": post_b,
            "w3T": w3T,
            "b3": b3,
        })
    return in_maps


def _unshard_output(results):
    out = np.empty((B, 2, PK, CK), dtype=np.float32)
    for core in range(N_CORES):
        b = core // CORES_PER_BATCH
        ps = (core % CORES_PER_BATCH) * P_SHARD
        out[b, :, ps:ps + P_SHARD, :] = results[core]["out"]
    return out


def kernel(x, primary_indices, compare_indices, pre_w, pre_b,
           post_w, post_b, post_out_w, post_out_b):
    in_maps = _shard_inputs(x, primary_indices, compare_indices, pre_w, pre_b,
                            post_w, post_b, post_out_w, post_out_b)
    nc = _get_nc()
    res = run_bass_kernel_spmd(nc, in_maps, core_ids=list(range(N_CORES)))
    return _unshard_output(res.results)


# revision 12
# speedup vs baseline: 25.0742x; 25.0742x over previous
"""Trainium2 Bass kernel for nn_DenseEdgeModel (gnn_message_passing).

Reference computation (all 1x1 convs == per-pixel matmuls over channels):
    h    = MLP3(x)                    # 3x (c->c) with ReLU between    [B,C,H,W]
    flat = h as [B*H*W, C]
    xp   = flat[primary_idx]          # [B,PK,C] -> [B,C,PK]
    xc   = flat[compare_idx]          # [B,CK,C] -> [B,C,CK]
    xx   = (xp[..,:,None]-xc[..,None,:])**2          # [B,C,PK,CK]
    g    = relu(W1@xx+b1); g = relu(W2@g+b2)         # over C
    out  = W3@g + b3                  # [B,2,PK,CK]

Sharding (8 cores): data-parallel over batch (4 cores per batch), and the
PK axis split 4-ways within each batch -> each core owns 64 primary
indices of one batch and all 256 compare indices of that batch.

Because the pre-MLP is per-pixel, gather commutes with it:
MLP(x)[idx] == MLP(x[idx]). The host therefore only *slices* (gathers
rows of x for each core's indices and transposes to channel-major) --
zero FLOPs on host; every matmul/activation runs on device.

Device kernel per core (fp32 data, float32r full-rate matmul streaming):
  stage 1: pre-MLP on the 320 gathered pixel columns  [c,320]
  stage 2: per pair of primary columns p ("group"): build xx tiles
           [c, 2*256] = (F[:,p]-F[:,64:320])^2 -- subtract on GPSIMD,
           square split ACT/DVE -- then the 2-layer post-MLP as PE
           matmuls (K=256 in 2 chunks, bias+relu split ACT/DVE), and a
           final (c->2) matmul quad-packed into one PSUM bank via
           tile_position col-tiling (bias pre-seeded by a K=1 matmul),
           drained by one DVE copy + 4 DMAs per quad.
"""

import numpy as np

import concourse.bass as bass
import concourse.tile as tile
from concourse import bacc, mybir
from concourse.bass_utils import run_bass_kernel_spmd

# Problem constants (hardcoded per harness contract)
B, C, H, W = 2, 256, 32, 32
PK, CK = 256, 256
N_CORES = 8
CORES_PER_BATCH = N_CORES // B          # 4
P_SHARD = PK // CORES_PER_BATCH         # 64 primary indices per core
NJ = P_SHARD + CK                       # 320 gathered pixel columns per core
PAIR = 2                                # primary columns per stage-2 group
NF = PAIR * CK                          # 512 = stage-2 matmul free dim
QUAD = 4                                # groups sharing one l3 PSUM bank
F32 = mybir.dt.float32
BF16 = mybir.dt.bfloat16                # layer-3 only (tile_position + f32r
                                        # are mutually exclusive in the ISA)
MM_DT = mybir.dt.float32r               # full-rate 4-byte matmul streaming
AF = mybir.ActivationFunctionType
OP = mybir.AluOpType
USE_GPSIMD_SUB = True                   # xx-gen subtract on GPSIMD


def _build_nc(reps=1):
    nc = bacc.Bacc("TRN2", target_bir_lowering=False, debug=False)

    xgT = nc.dram_tensor("xgT", [C, NJ], F32, kind="ExternalInput")
    pre_wT = nc.dram_tensor("pre_wT", [3, C, C], F32, kind="ExternalInput")
    pre_b = nc.dram_tensor("pre_b", [3, C], F32, kind="ExternalInput")
    post_wT = nc.dram_tensor("post_wT", [2, C, C], F32, kind="ExternalInput")
    post_b = nc.dram_tensor("post_b", [2, C], F32, kind="ExternalInput")
    w3T = nc.dram_tensor("w3T", [C, 2], BF16, kind="ExternalInput")
    # output bias as a [1,128] row: b3[j] at col 32*k+j, used as K=1 lhsT to
    # seed the quad-packed layer-3 PSUM bank (host-built, zero FLOPs)
    b3row_d = nc.dram_tensor("b3row", [1, 128], F32, kind="ExternalInput")
    ones_d = nc.dram_tensor("ones_row", [1, NF], F32, kind="ExternalInput")
    out = nc.dram_tensor("out", [2, P_SHARD, CK], F32, kind="ExternalOutput")

    with tile.TileContext(nc) as tc:
        for _ in range(reps):
            _kernel_body(nc, tc, xgT, pre_wT, pre_b, post_wT, post_b, w3T,
                         b3row_d, ones_d, out)
    nc.compile()
    return nc


def _kernel_body(nc, tc, xgT, pre_wT, pre_b, post_wT, post_b, w3T, b3row_d,
                 ones_d, out):
    with (
        tc.tile_pool(name="consts", bufs=1) as consts,
        tc.tile_pool(name="feats", bufs=1) as feats,
        tc.tile_pool(name="work", bufs=3) as work,
        tc.tile_pool(name="psum", bufs=1, space="PSUM") as psum,
    ):
        # ---- weights / biases to SBUF (replicated, loaded once) ----
        def col(ap_1d):  # [n] -> [n,1]
            return ap_1d.rearrange("(n one) -> n one", one=1)

        wpre = [[consts.tile([128, C], MM_DT, name=f"wpre_{i}_{cc}")
                 for cc in range(2)] for i in range(3)]
        bpre = [[consts.tile([128, 1], F32, name=f"bpre_{i}_{oc}")
                 for oc in range(2)] for i in range(3)]
        for i in range(3):
            for cc in range(2):
                nc.sync.dma_start(
                    out=wpre[i][cc][:],
                    in_=pre_wT[i, cc * 128:(cc + 1) * 128, :].bitcast(MM_DT))
            for oc in range(2):
                nc.sync.dma_start(out=bpre[i][oc][:],
                                  in_=col(pre_b[i, oc * 128:(oc + 1) * 128]))
        wpost = [[consts.tile([128, C], MM_DT, name=f"wpost_{l}_{cc}")
                  for cc in range(2)] for l in range(2)]
        bpost = [[consts.tile([128, 1], F32, name=f"bpost_{l}_{oc}")
                  for oc in range(2)] for l in range(2)]
        for l in range(2):
            for cc in range(2):
                nc.sync.dma_start(
                    out=wpost[l][cc][:],
                    in_=post_wT[l, cc * 128:(cc + 1) * 128, :].bitcast(MM_DT))
            for oc in range(2):
                nc.sync.dma_start(out=bpost[l][oc][:],
                                  in_=col(post_b[l, oc * 128:(oc + 1) * 128]))
        w3 = [consts.tile([128, 2], BF16, name=f"w3_{cc}") for cc in range(2)]
        for cc in range(2):
            nc.sync.dma_start(out=w3[cc][:],
                              in_=w3T[cc * 128:(cc + 1) * 128, :])
        b3row = consts.tile([1, 128], MM_DT, name="b3row")
        nc.sync.dma_start(out=b3row[:], in_=b3row_d.ap().bitcast(MM_DT))
        ones_row = consts.tile([1, NF], MM_DT, name="ones_row")
        nc.sync.dma_start(out=ones_row[:], in_=ones_d.ap().bitcast(MM_DT))

        # ---- stage 1: pre-MLP over the 320 gathered pixel columns ----
        cur = [feats.tile([128, NJ], MM_DT, name=f"xg_{cc}") for cc in range(2)]
        for cc in range(2):
            nc.sync.dma_start(out=cur[cc][:],
                              in_=xgT[cc * 128:(cc + 1) * 128, :].bitcast(MM_DT))
        for i in range(3):
            nxt = [feats.tile([128, NJ], MM_DT if i < 2 else F32,
                              name=f"feat{i}_{oc}") for oc in range(2)]
            for oc in range(2):
                ps = psum.tile([128, NJ], F32, name=f"ps_s1_{i}_{oc}",
                               tag="ps_a", bufs=3)
                nc.tensor.matmul(
                    out=ps[:],
                    lhsT=wpre[i][0][:, oc * 128:(oc + 1) * 128],
                    rhs=cur[0][:], start=True, stop=False)
                nc.tensor.matmul(
                    out=ps[:],
                    lhsT=wpre[i][1][:, oc * 128:(oc + 1) * 128],
                    rhs=cur[1][:], start=False, stop=True)
                nc.scalar.activation(
                    out=nxt[oc][:], in_=ps[:],
                    func=AF.Relu if i < 2 else AF.Identity,
                    bias=bpre[i][oc][:, 0:1], scale=1.0)
            cur = nxt
        F = cur  # [128, NJ] x2 chunks; cols 0:P_SHARD primary, rest compare

        # ---- stage 2: software-pipelined with a 3-deep skew ----
        # Emitting all stages of one group back-to-back serializes on the
        # xx -> l1 -> relu -> l2 -> relu -> l3 chain. Instead, iteration i
        # emits l3(i-3), l2(i-2), l1(i-1), xx(i): every engine's program
        # order then interleaves groups whose dependencies are already in
        # flight, so PE never waits on the same group's elementwise ops.
        NG = P_SHARD // PAIR
        xx_t, h0_t, h1_t, ps3_t = {}, {}, {}, {}

        def stage_xx(g):
            xx = [work.tile([128, NF], MM_DT, name=f"xx_{cc}", tag=f"xx_{cc}")
                  for cc in range(2)]
            for s in range(PAIR):
                p = g * PAIR + s
                for cc in range(2):
                    dst = xx[cc][:, s * CK:(s + 1) * CK]
                    if USE_GPSIMD_SUB:
                        d = work.tile([128, CK], F32, name=f"d_{cc}_{s}",
                                      tag=f"d_{cc}_{s}")
                        nc.gpsimd.tensor_scalar(
                            out=d[:], in0=F[cc][:, P_SHARD:NJ],
                            scalar1=F[cc][:, p:p + 1], scalar2=None,
                            op0=OP.subtract)
                        if (s + cc) % 2 == 0:
                            nc.scalar.activation(out=dst, in_=d[:],
                                                 func=AF.Square)
                        else:
                            nc.vector.tensor_tensor(out=dst, in0=d[:],
                                                    in1=d[:], op=OP.mult)
                    else:
                        nc.scalar.activation(
                            out=dst, in_=F[cc][:, P_SHARD:NJ],
                            func=AF.Square,
                            bias=F[cc][:, p:p + 1], scale=-1.0)
            xx_t[g] = xx

        def stage_mlp(g, l, hcur, out_map):
            odt = MM_DT if l == 0 else BF16
            hnxt = [work.tile([128, NF], odt, name=f"h{l}_{oc}",
                              tag=f"h{l}_{oc}") for oc in range(2)]
            for oc in range(2):
                ps = psum.tile([128, NF], F32, name=f"ps_l{l}_{oc}",
                               tag="ps_a" if l == 0 else "ps_b", bufs=3)
                nc.tensor.matmul(
                    out=ps[:],
                    lhsT=wpost[l][0][:, oc * 128:(oc + 1) * 128],
                    rhs=hcur[0][:], start=True, stop=False)
                nc.tensor.matmul(
                    out=ps[:],
                    lhsT=wpost[l][1][:, oc * 128:(oc + 1) * 128],
                    rhs=hcur[1][:], start=False, stop=True)
                if (l + oc) % 2 == 0:
                    nc.scalar.activation(
                        out=hnxt[oc][:], in_=ps[:], func=AF.Relu,
                        bias=bpost[l][oc][:, 0:1], scale=1.0)
                else:
                    nc.vector.tensor_scalar(
                        out=hnxt[oc][:], in0=ps[:],
                        scalar1=bpost[l][oc][:, 0:1], scalar2=0.0,
                        op0=OP.add, op1=OP.max)
            out_map[g] = hnxt

        def stage_l3(g):
            # layer 3 (c->2): quad-packed into one PSUM bank -- group g%QUAD
            # computes at array col-group k, writing PSUM partitions
            # 32k..32k+1. Bias pre-seeded by a K=1 matmul that defines every
            # row. One DVE copy + 4 DMAs drain the quad.
            hcur = h1_t.pop(g)
            k = g % QUAD
            if k == 0:
                ps3 = psum.tile([128, NF], F32, name="ps3", tag="ps3", bufs=2)
                nc.tensor.matmul(out=ps3[:], lhsT=b3row[:], rhs=ones_row[:],
                                 start=True, stop=True)
                ps3_t[g // QUAD] = ps3
            ps3 = ps3_t[g // QUAD]
            nc.tensor.matmul(out=ps3[32 * k:32 * k + 2, :], lhsT=w3[0][:],
                             rhs=hcur[0][:], tile_position=(0, 32 * k),
                             start=False, stop=True, skip_group_check=True)
            nc.tensor.matmul(out=ps3[32 * k:32 * k + 2, :], lhsT=w3[1][:],
                             rhs=hcur[1][:], tile_position=(0, 32 * k),
                             start=False, stop=True, skip_group_check=True)
            if k == QUAD - 1:
                ps3_t.pop(g // QUAD)
                ob = work.tile([128, NF], F32, name="ob", tag="ob", bufs=2)
                nc.vector.tensor_copy(out=ob[:], in_=ps3[:])
                qb = (g // QUAD) * QUAD * PAIR
                for kk in range(QUAD):
                    nc.sync.dma_start(
                        out=out[:, qb + kk * PAIR:qb + (kk + 1) * PAIR, :],
                        in_=ob[32 * kk:32 * kk + 2, :].rearrange(
                            "j (s q) -> j s q", s=PAIR))

        for i in range(NG + 3):
            if i >= 3:
                stage_l3(i - 3)
            if 2 <= i < NG + 2:
                stage_mlp(i - 2, 1, h0_t.pop(i - 2), h1_t)
            if 1 <= i < NG + 1:
                stage_mlp(i - 1, 0, xx_t.pop(i - 1), h0_t)
            if i < NG:
                stage_xx(i)


_NC_CACHE = {}


def _get_nc():
    if "nc" not in _NC_CACHE:
        _NC_CACHE["nc"] = _build_nc()
    return _NC_CACHE["nc"]


def _shard_inputs(x, primary_indices, compare_indices, pre_w, pre_b,
                  post_w, post_b, post_out_w, post_out_b):
    """Host-side sharding: per-core index slicing + row gather of x (pure
    data movement -- the pre-MLP commutes with the gather) and weight
    transposes. Returns the 8 per-core input maps."""
    x = np.asarray(x, dtype=np.float32)
    x_rows = np.ascontiguousarray(x.transpose(0, 2, 3, 1)).reshape(B * H * W, C)
    pre_wT = np.ascontiguousarray(
        np.asarray(pre_w, dtype=np.float32).transpose(0, 2, 1))
    post_wT = np.ascontiguousarray(
        np.asarray(post_w, dtype=np.float32).transpose(0, 2, 1))
    import ml_dtypes
    w3T = np.ascontiguousarray(
        np.asarray(post_out_w, dtype=np.float32).T).astype(ml_dtypes.bfloat16)
    pre_b = np.ascontiguousarray(np.asarray(pre_b, dtype=np.float32))
    post_b = np.ascontiguousarray(np.asarray(post_b, dtype=np.float32))
    b3 = np.asarray(post_out_b, dtype=np.float32)
    b3row = np.zeros((1, 128), dtype=np.float32)
    for k in range(QUAD):
        b3row[0, 32 * k:32 * k + 2] = b3
    primary_indices = np.asarray(primary_indices)
    compare_indices = np.asarray(compare_indices)

    in_maps = []
    for core in range(N_CORES):
        b = core // CORES_PER_BATCH
        ps = (core % CORES_PER_BATCH) * P_SHARD
        rows = np.concatenate([
            primary_indices[b, ps:ps + P_SHARD].astype(np.int64),
            compare_indices[b].astype(np.int64),
        ])
        xg_T = np.ascontiguousarray(x_rows[rows].T)  # [C, NJ]
        in_maps.append({
            "xgT": xg_T,
            "pre_wT": pre_wT,
            "pre_b": pre_b,
            "post_wT": post_wT,
            "post_b": post_b,
            "w3T": w3T,
            "b3row": b3row,
            "ones_row": np.ones((1, NF), dtype=np.float32),
        })
    return in_maps


def _unshard_output(results):
    out = np.empty((B, 2, PK, CK), dtype=np.float32)
    for core in range(N_CORES):
        b = core // CORES_PER_BATCH
        ps = (core % CORES_PER_BATCH) * P_SHARD
        out[b, :, ps:ps + P_SHARD, :] = results[core]["out"]
    return out


def kernel(x, primary_indices, compare_indices, pre_w, pre_b,
           post_w, post_b, post_out_w, post_out_b):
    in_maps = _shard_inputs(x, primary_indices, compare_indices, pre_w, pre_b,
                            post_w, post_b, post_out_w, post_out_b)
    nc = _get_nc()
    res = run_bass_kernel_spmd(nc, in_maps, core_ids=list(range(N_CORES)))
    return _unshard_output(res.results)


# revision 13
# speedup vs baseline: 25.1822x; 1.0043x over previous
"""Trainium2 Bass kernel for nn_DenseEdgeModel (gnn_message_passing).

Reference computation (all 1x1 convs == per-pixel matmuls over channels):
    h    = MLP3(x)                    # 3x (c->c) with ReLU between    [B,C,H,W]
    flat = h as [B*H*W, C]
    xp   = flat[primary_idx]          # [B,PK,C] -> [B,C,PK]
    xc   = flat[compare_idx]          # [B,CK,C] -> [B,C,CK]
    xx   = (xp[..,:,None]-xc[..,None,:])**2          # [B,C,PK,CK]
    g    = relu(W1@xx+b1); g = relu(W2@g+b2)         # over C
    out  = W3@g + b3                  # [B,2,PK,CK]

Sharding (8 cores): data-parallel over batch (4 cores per batch), and the
PK axis split 4-ways within each batch -> each core owns 64 primary
indices of one batch and all 256 compare indices of that batch.

Because the pre-MLP is per-pixel, gather commutes with it:
MLP(x)[idx] == MLP(x[idx]). The host therefore only *slices* (gathers
rows of x for each core's indices and transposes to channel-major) --
zero FLOPs on host; every matmul/activation runs on device.

Device kernel per core (fp32 data, float32r full-rate matmul streaming):
  stage 1: pre-MLP on the 320 gathered pixel columns  [c,320]
  stage 2: per pair of primary columns p ("group"): build xx tiles
           [c, 2*256] = (F[:,p]-F[:,64:320])^2 -- subtract on GPSIMD,
           square split ACT/DVE -- then the 2-layer post-MLP as PE
           matmuls (K=256 in 2 chunks, bias+relu split ACT/DVE), and a
           final (c->2) matmul quad-packed into one PSUM bank via
           tile_position col-tiling (bias pre-seeded by a K=1 matmul),
           drained by one DVE copy + 4 DMAs per quad.
"""

import numpy as np

import concourse.bass as bass
import concourse.tile as tile
from concourse import bacc, mybir
from concourse.bass_utils import run_bass_kernel_spmd

# Problem constants (hardcoded per harness contract)
B, C, H, W = 2, 256, 32, 32
PK, CK = 256, 256
N_CORES = 8
CORES_PER_BATCH = N_CORES // B          # 4
P_SHARD = PK // CORES_PER_BATCH         # 64 primary indices per core
NJ = P_SHARD + CK                       # 320 gathered pixel columns per core
PAIR = 2                                # primary columns per stage-2 group
NF = PAIR * CK                          # 512 = stage-2 matmul free dim
QUAD = 4                                # groups sharing one l3 PSUM bank
F32 = mybir.dt.float32
BF16 = mybir.dt.bfloat16                # layer-3 only (tile_position + f32r
                                        # are mutually exclusive in the ISA)
MM_DT = mybir.dt.float32r               # full-rate 4-byte matmul streaming
AF = mybir.ActivationFunctionType
OP = mybir.AluOpType
USE_GPSIMD_SUB = False                   # xx-gen subtract on GPSIMD


def _build_nc(reps=1):
    nc = bacc.Bacc("TRN2", target_bir_lowering=False, debug=False)

    xgT = nc.dram_tensor("xgT", [C, NJ], F32, kind="ExternalInput")
    pre_wT = nc.dram_tensor("pre_wT", [3, C, C], F32, kind="ExternalInput")
    pre_b = nc.dram_tensor("pre_b", [3, C], F32, kind="ExternalInput")
    post_wT = nc.dram_tensor("post_wT", [2, C, C], F32, kind="ExternalInput")
    post_b = nc.dram_tensor("post_b", [2, C], F32, kind="ExternalInput")
    w3T = nc.dram_tensor("w3T", [C, 2], BF16, kind="ExternalInput")
    # output bias as a [1,128] row: b3[j] at col 32*k+j, used as K=1 lhsT to
    # seed the quad-packed layer-3 PSUM bank (host-built, zero FLOPs)
    b3row_d = nc.dram_tensor("b3row", [1, 128], F32, kind="ExternalInput")
    ones_d = nc.dram_tensor("ones_row", [1, NF], F32, kind="ExternalInput")
    out = nc.dram_tensor("out", [2, P_SHARD, CK], F32, kind="ExternalOutput")

    with tile.TileContext(nc) as tc:
        for _ in range(reps):
            _kernel_body(nc, tc, xgT, pre_wT, pre_b, post_wT, post_b, w3T,
                         b3row_d, ones_d, out)
    nc.compile()
    return nc


def _kernel_body(nc, tc, xgT, pre_wT, pre_b, post_wT, post_b, w3T, b3row_d,
                 ones_d, out):
    with (
        tc.tile_pool(name="consts", bufs=1) as consts,
        tc.tile_pool(name="feats", bufs=1) as feats,
        tc.tile_pool(name="work", bufs=3) as work,
        tc.tile_pool(name="psum", bufs=1, space="PSUM") as psum,
    ):
        # ---- weights / biases to SBUF (replicated, loaded once) ----
        def col(ap_1d):  # [n] -> [n,1]
            return ap_1d.rearrange("(n one) -> n one", one=1)

        wpre = [[consts.tile([128, C], MM_DT, name=f"wpre_{i}_{cc}")
                 for cc in range(2)] for i in range(3)]
        bpre = [[consts.tile([128, 1], F32, name=f"bpre_{i}_{oc}")
                 for oc in range(2)] for i in range(3)]
        for i in range(3):
            for cc in range(2):
                nc.sync.dma_start(
                    out=wpre[i][cc][:],
                    in_=pre_wT[i, cc * 128:(cc + 1) * 128, :].bitcast(MM_DT))
            for oc in range(2):
                nc.sync.dma_start(out=bpre[i][oc][:],
                                  in_=col(pre_b[i, oc * 128:(oc + 1) * 128]))
        wpost = [[consts.tile([128, C], MM_DT, name=f"wpost_{l}_{cc}")
                  for cc in range(2)] for l in range(2)]
        bpost = [[consts.tile([128, 1], F32, name=f"bpost_{l}_{oc}")
                  for oc in range(2)] for l in range(2)]
        for l in range(2):
            for cc in range(2):
                nc.sync.dma_start(
                    out=wpost[l][cc][:],
                    in_=post_wT[l, cc * 128:(cc + 1) * 128, :].bitcast(MM_DT))
            for oc in range(2):
                nc.sync.dma_start(out=bpost[l][oc][:],
                                  in_=col(post_b[l, oc * 128:(oc + 1) * 128]))
        w3 = [consts.tile([128, 2], BF16, name=f"w3_{cc}") for cc in range(2)]
        for cc in range(2):
            nc.sync.dma_start(out=w3[cc][:],
                              in_=w3T[cc * 128:(cc + 1) * 128, :])
        b3row = consts.tile([1, 128], MM_DT, name="b3row")
        nc.sync.dma_start(out=b3row[:], in_=b3row_d.ap().bitcast(MM_DT))
        ones_row = consts.tile([1, NF], MM_DT, name="ones_row")
        nc.sync.dma_start(out=ones_row[:], in_=ones_d.ap().bitcast(MM_DT))

        # ---- stage 1: pre-MLP over the 320 gathered pixel columns ----
        cur = [feats.tile([128, NJ], MM_DT, name=f"xg_{cc}") for cc in range(2)]
        for cc in range(2):
            nc.sync.dma_start(out=cur[cc][:],
                              in_=xgT[cc * 128:(cc + 1) * 128, :].bitcast(MM_DT))
        for i in range(3):
            nxt = [feats.tile([128, NJ], MM_DT if i < 2 else F32,
                              name=f"feat{i}_{oc}") for oc in range(2)]
            for oc in range(2):
                ps = psum.tile([128, NJ], F32, name=f"ps_s1_{i}_{oc}",
                               tag="ps_a", bufs=3)
                nc.tensor.matmul(
                    out=ps[:],
                    lhsT=wpre[i][0][:, oc * 128:(oc + 1) * 128],
                    rhs=cur[0][:], start=True, stop=False)
                nc.tensor.matmul(
                    out=ps[:],
                    lhsT=wpre[i][1][:, oc * 128:(oc + 1) * 128],
                    rhs=cur[1][:], start=False, stop=True)
                nc.scalar.activation(
                    out=nxt[oc][:], in_=ps[:],
                    func=AF.Relu if i < 2 else AF.Identity,
                    bias=bpre[i][oc][:, 0:1], scale=1.0)
            cur = nxt
        F = cur  # [128, NJ] x2 chunks; cols 0:P_SHARD primary, rest compare

        # ---- stage 2: software-pipelined with a 3-deep skew ----
        # Emitting all stages of one group back-to-back serializes on the
        # xx -> l1 -> relu -> l2 -> relu -> l3 chain. Instead, iteration i
        # emits l3(i-3), l2(i-2), l1(i-1), xx(i): every engine's program
        # order then interleaves groups whose dependencies are already in
        # flight, so PE never waits on the same group's elementwise ops.
        NG = P_SHARD // PAIR
        xx_t, h0_t, h1_t, ps3_t = {}, {}, {}, {}

        def stage_xx(g):
            xx = [work.tile([128, NF], MM_DT, name=f"xx_{cc}", tag=f"xx_{cc}")
                  for cc in range(2)]
            for s in range(PAIR):
                p = g * PAIR + s
                for cc in range(2):
                    dst = xx[cc][:, s * CK:(s + 1) * CK]
                    if USE_GPSIMD_SUB:
                        d = work.tile([128, CK], F32, name=f"d_{cc}_{s}",
                                      tag=f"d_{cc}_{s}")
                        nc.gpsimd.tensor_scalar(
                            out=d[:], in0=F[cc][:, P_SHARD:NJ],
                            scalar1=F[cc][:, p:p + 1], scalar2=None,
                            op0=OP.subtract)
                        if (s + cc) % 2 == 0:
                            nc.scalar.activation(out=dst, in_=d[:],
                                                 func=AF.Square)
                        else:
                            nc.vector.tensor_tensor(out=dst, in0=d[:],
                                                    in1=d[:], op=OP.mult)
                    else:
                        nc.scalar.activation(
                            out=dst, in_=F[cc][:, P_SHARD:NJ],
                            func=AF.Square,
                            bias=F[cc][:, p:p + 1], scale=-1.0)
            xx_t[g] = xx

        def stage_mlp(g, l, hcur, out_map):
            odt = MM_DT if l == 0 else BF16
            hnxt = [work.tile([128, NF], odt, name=f"h{l}_{oc}",
                              tag=f"h{l}_{oc}") for oc in range(2)]
            for oc in range(2):
                ps = psum.tile([128, NF], F32, name=f"ps_l{l}_{oc}",
                               tag="ps_a" if l == 0 else "ps_b", bufs=3)
                nc.tensor.matmul(
                    out=ps[:],
                    lhsT=wpost[l][0][:, oc * 128:(oc + 1) * 128],
                    rhs=hcur[0][:], start=True, stop=False)
                nc.tensor.matmul(
                    out=ps[:],
                    lhsT=wpost[l][1][:, oc * 128:(oc + 1) * 128],
                    rhs=hcur[1][:], start=False, stop=True)
                if (l + oc) % 2 == 0:
                    nc.scalar.activation(
                        out=hnxt[oc][:], in_=ps[:], func=AF.Relu,
                        bias=bpost[l][oc][:, 0:1], scale=1.0)
                else:
                    nc.vector.tensor_scalar(
                        out=hnxt[oc][:], in0=ps[:],
                        scalar1=bpost[l][oc][:, 0:1], scalar2=0.0,
                        op0=OP.add, op1=OP.max)
            out_map[g] = hnxt

        def stage_l3(g):
            # layer 3 (c->2): quad-packed into one PSUM bank -- group g%QUAD
            # computes at array col-group k, writing PSUM partitions
            # 32k..32k+1. Bias pre-seeded by a K=1 matmul that defines every
            # row. One DVE copy + 4 DMAs drain the quad.
            hcur = h1_t.pop(g)
            k = g % QUAD
            if k == 0:
                ps3 = psum.tile([128, NF], F32, name="ps3", tag="ps3", bufs=2)
                nc.tensor.matmul(out=ps3[:], lhsT=b3row[:], rhs=ones_row[:],
                                 start=True, stop=True)
                ps3_t[g // QUAD] = ps3
            ps3 = ps3_t[g // QUAD]
            nc.tensor.matmul(out=ps3[32 * k:32 * k + 2, :], lhsT=w3[0][:],
                             rhs=hcur[0][:], tile_position=(0, 32 * k),
                             start=False, stop=True, skip_group_check=True)
            nc.tensor.matmul(out=ps3[32 * k:32 * k + 2, :], lhsT=w3[1][:],
                             rhs=hcur[1][:], tile_position=(0, 32 * k),
                             start=False, stop=True, skip_group_check=True)
            if k == QUAD - 1:
                ps3_t.pop(g // QUAD)
                ob = work.tile([128, NF], F32, name="ob", tag="ob", bufs=2)
                nc.vector.tensor_copy(out=ob[:], in_=ps3[:])
                qb = (g // QUAD) * QUAD * PAIR
                for kk in range(QUAD):
                    nc.sync.dma_start(
                        out=out[:, qb + kk * PAIR:qb + (kk + 1) * PAIR, :],
                        in_=ob[32 * kk:32 * kk + 2, :].rearrange(
                            "j (s q) -> j s q", s=PAIR))

        for i in range(NG + 3):
            if i >= 3:
                stage_l3(i - 3)
            if 2 <= i < NG + 2:
                stage_mlp(i - 2, 1, h0_t.pop(i - 2), h1_t)
            if 1 <= i < NG + 1:
                stage_mlp(i - 1, 0, xx_t.pop(i - 1), h0_t)
            if i < NG:
                stage_xx(i)


_NC_CACHE = {}


def _get_nc():
    if "nc" not in _NC_CACHE:
        _NC_CACHE["nc"] = _build_nc()
    return _NC_CACHE["nc"]


def _shard_inputs(x, primary_indices, compare_indices, pre_w, pre_b,
                  post_w, post_b, post_out_w, post_out_b):
    """Host-side sharding: per-core index slicing + row gather of x (pure
    data movement -- the pre-MLP commutes with the gather) and weight
    transposes. Returns the 8 per-core input maps."""
    x = np.asarray(x, dtype=np.float32)
    x_rows = np.ascontiguousarray(x.transpose(0, 2, 3, 1)).reshape(B * H * W, C)
    pre_wT = np.ascontiguousarray(
        np.asarray(pre_w, dtype=np.float32).transpose(0, 2, 1))
    post_wT = np.ascontiguousarray(
        np.asarray(post_w, dtype=np.float32).transpose(0, 2, 1))
    import ml_dtypes
    w3T = np.ascontiguousarray(
        np.asarray(post_out_w, dtype=np.float32).T).astype(ml_dtypes.bfloat16)
    pre_b = np.ascontiguousarray(np.asarray(pre_b, dtype=np.float32))
    post_b = np.ascontiguousarray(np.asarray(post_b, dtype=np.float32))
    b3 = np.asarray(post_out_b, dtype=np.float32)
    b3row = np.zeros((1, 128), dtype=np.float32)
    for k in range(QUAD):
        b3row[0, 32 * k:32 * k + 2] = b3
    primary_indices = np.asarray(primary_indices)
    compare_indices = np.asarray(compare_indices)

    in_maps = []
    for core in range(N_CORES):
        b = core // CORES_PER_BATCH
        ps = (core % CORES_PER_BATCH) * P_SHARD
        rows = np.concatenate([
            primary_indices[b, ps:ps + P_SHARD].astype(np.int64),
            compare_indices[b].astype(np.int64),
        ])
        xg_T = np.ascontiguousarray(x_rows[rows].T)  # [C, NJ]
        in_maps.append({
            "xgT": xg_T,
            "pre_wT": pre_wT,
            "pre_b": pre_b,
            "post_wT": post_wT,
            "post_b": post_b,
            "w3T": w3T,
            "b3row": b3row,
            "ones_row": np.ones((1, NF), dtype=np.float32),
        })
    return in_maps


def _unshard_output(results):
    out = np.empty((B, 2, PK, CK), dtype=np.float32)
    for core in range(N_CORES):
        b = core // CORES_PER_BATCH
        ps = (core % CORES_PER_BATCH) * P_SHARD
        out[b, :, ps:ps + P_SHARD, :] = results[core]["out"]
    return out


def kernel(x, primary_indices, compare_indices, pre_w, pre_b,
           post_w, post_b, post_out_w, post_out_b):
    in_maps = _shard_inputs(x, primary_indices, compare_indices, pre_w, pre_b,
                            post_w, post_b, post_out_w, post_out_b)
    nc = _get_nc()
    res = run_bass_kernel_spmd(nc, in_maps, core_ids=list(range(N_CORES)))
    return _unshard_output(res.results)


# revision 16
# speedup vs baseline: 136.0084x; 5.4010x over previous
"""Trainium2 Bass kernel for nn_DenseEdgeModel (gnn_message_passing).

Reference computation (all 1x1 convs == per-pixel matmuls over channels):
    h    = MLP3(x)                    # 3x (c->c) with ReLU between    [B,C,H,W]
    flat = h as [B*H*W, C]
    xp   = flat[primary_idx]          # [B,PK,C] -> [B,C,PK]
    xc   = flat[compare_idx]          # [B,CK,C] -> [B,C,CK]
    xx   = (xp[..,:,None]-xc[..,None,:])**2          # [B,C,PK,CK]
    g    = relu(W1@xx+b1); g = relu(W2@g+b2)         # over C
    out  = W3@g + b3                  # [B,2,PK,CK]

Sharding (8 cores): data-parallel over batch (4 cores per batch), and the
PK axis split 4-ways within each batch -> each core owns 64 primary
indices of one batch and all 256 compare indices of that batch.

Because the pre-MLP is per-pixel, gather commutes with it:
MLP(x)[idx] == MLP(x[idx]). The host therefore only *slices* (gathers
rows of x for each core's indices and transposes to channel-major) --
zero FLOPs on host; every matmul/activation runs on device.

Device kernel per core (fp32 data, float32r full-rate matmul streaming):
  stage 1: pre-MLP on the 320 gathered pixel columns  [c,320]
  stage 2: per pair of primary columns p ("group"): build xx tiles
           [c, 2*256] = (F[:,p]-F[:,64:320])^2 -- subtract on GPSIMD,
           square split ACT/DVE -- then the 2-layer post-MLP as PE
           matmuls (K=256 in 2 chunks, bias+relu split ACT/DVE), and a
           final (c->2) matmul quad-packed into one PSUM bank via
           tile_position col-tiling (bias pre-seeded by a K=1 matmul),
           drained by one DVE copy + 4 DMAs per quad.
"""

import os

import numpy as np

import concourse.bass as bass
import concourse.tile as tile
from concourse import bacc, mybir
from concourse.bass_utils import run_bass_kernel_spmd

# Problem constants (hardcoded per harness contract)
B, C, H, W = 2, 256, 32, 32
PK, CK = 256, 256
N_CORES = 8
CORES_PER_BATCH = N_CORES // B          # 4
P_SHARD = PK // CORES_PER_BATCH         # 64 primary indices per core
NJ = P_SHARD + CK                       # 320 gathered pixel columns per core
PAIR = 2                                # primary columns per stage-2 group
NF = PAIR * CK                          # 512 = stage-2 matmul free dim
QUAD = 4                                # groups sharing one l3 PSUM bank
F32 = mybir.dt.float32
BF16 = mybir.dt.bfloat16                # layer-3 only (tile_position + f32r
                                        # are mutually exclusive in the ISA)
MM_DT = mybir.dt.float32r               # full-rate 4-byte matmul streaming
VARIANT = os.environ.get("KERNEL_VARIANT", "")
AF = mybir.ActivationFunctionType
OP = mybir.AluOpType
USE_GPSIMD_SUB = False                   # xx-gen subtract on GPSIMD


def _build_nc(reps=1):
    nc = bacc.Bacc("TRN2", target_bir_lowering=False, debug=False)

    xgT = nc.dram_tensor("xgT", [C, NJ], F32, kind="ExternalInput")
    pre_wT = nc.dram_tensor("pre_wT", [3, C, C], F32, kind="ExternalInput")
    pre_b = nc.dram_tensor("pre_b", [3, C], F32, kind="ExternalInput")
    post_wT = nc.dram_tensor("post_wT", [2, C, C], F32, kind="ExternalInput")
    post_b = nc.dram_tensor("post_b", [2, C], F32, kind="ExternalInput")
    w3T = nc.dram_tensor("w3T", [C, 2], BF16, kind="ExternalInput")
    # output bias as a [1,128] row: b3[j] at col 32*k+j, used as K=1 lhsT to
    # seed the quad-packed layer-3 PSUM bank (host-built, zero FLOPs)
    b3row_d = nc.dram_tensor("b3row", [1, 128], F32, kind="ExternalInput")
    ones_d = nc.dram_tensor("ones_row", [1, NF], F32, kind="ExternalInput")
    # reps > 1 (timing builds): each rep writes its own output slice so no
    # rep's stores are dead
    oshape = [2, P_SHARD, CK] if reps == 1 else [reps, 2, P_SHARD, CK]
    out = nc.dram_tensor("out", oshape, F32, kind="ExternalOutput")

    with tile.TileContext(nc) as tc:
        for r in range(reps):
            out_r = out.ap() if reps == 1 else out[r]
            _kernel_body(nc, tc, xgT, pre_wT, pre_b, post_wT, post_b, w3T,
                         b3row_d, ones_d, out_r)
    nc.compile()
    return nc


def _kernel_body(nc, tc, xgT, pre_wT, pre_b, post_wT, post_b, w3T, b3row_d,
                 ones_d, out):
    with (
        tc.tile_pool(name="consts", bufs=1) as consts,
        tc.tile_pool(name="feats", bufs=1) as feats,
        tc.tile_pool(name="work", bufs=3) as work,
        tc.tile_pool(name="psum", bufs=1, space="PSUM") as psum,
    ):
        # ---- weights / biases to SBUF (replicated, loaded once) ----
        def col(ap_1d):  # [n] -> [n,1]
            return ap_1d.rearrange("(n one) -> n one", one=1)

        wpre = [[consts.tile([128, C], MM_DT, name=f"wpre_{i}_{cc}")
                 for cc in range(2)] for i in range(3)]
        bpre = [[consts.tile([128, 1], F32, name=f"bpre_{i}_{oc}")
                 for oc in range(2)] for i in range(3)]
        for i in range(3):
            for cc in range(2):
                nc.sync.dma_start(
                    out=wpre[i][cc][:],
                    in_=pre_wT[i, cc * 128:(cc + 1) * 128, :].bitcast(MM_DT))
            for oc in range(2):
                nc.sync.dma_start(out=bpre[i][oc][:],
                                  in_=col(pre_b[i, oc * 128:(oc + 1) * 128]))
        wpost = [[consts.tile([128, C], MM_DT, name=f"wpost_{l}_{cc}")
                  for cc in range(2)] for l in range(2)]
        bpost = [[consts.tile([128, 1], F32, name=f"bpost_{l}_{oc}")
                  for oc in range(2)] for l in range(2)]
        for l in range(2):
            for cc in range(2):
                nc.sync.dma_start(
                    out=wpost[l][cc][:],
                    in_=post_wT[l, cc * 128:(cc + 1) * 128, :].bitcast(MM_DT))
            for oc in range(2):
                nc.sync.dma_start(out=bpost[l][oc][:],
                                  in_=col(post_b[l, oc * 128:(oc + 1) * 128]))
        w3 = [consts.tile([128, 2], BF16, name=f"w3_{cc}") for cc in range(2)]
        for cc in range(2):
            nc.sync.dma_start(out=w3[cc][:],
                              in_=w3T[cc * 128:(cc + 1) * 128, :])
        b3row = consts.tile([1, 128], MM_DT, name="b3row")
        nc.sync.dma_start(out=b3row[:], in_=b3row_d.ap().bitcast(MM_DT))
        ones_row = consts.tile([1, NF], MM_DT, name="ones_row")
        nc.sync.dma_start(out=ones_row[:], in_=ones_d.ap().bitcast(MM_DT))

        # ---- stage 1: pre-MLP over the 320 gathered pixel columns ----
        cur = [feats.tile([128, NJ], MM_DT, name=f"xg_{cc}") for cc in range(2)]
        for cc in range(2):
            nc.sync.dma_start(out=cur[cc][:],
                              in_=xgT[cc * 128:(cc + 1) * 128, :].bitcast(MM_DT))
        for i in range(3):
            nxt = [feats.tile([128, NJ], MM_DT if i < 2 else F32,
                              name=f"feat{i}_{oc}") for oc in range(2)]
            for oc in range(2):
                ps = psum.tile([128, NJ], F32, name=f"ps_s1_{i}_{oc}",
                               tag="ps_a", bufs=3)
                nc.tensor.matmul(
                    out=ps[:],
                    lhsT=wpre[i][0][:, oc * 128:(oc + 1) * 128],
                    rhs=cur[0][:], start=True, stop=False)
                nc.tensor.matmul(
                    out=ps[:],
                    lhsT=wpre[i][1][:, oc * 128:(oc + 1) * 128],
                    rhs=cur[1][:], start=False, stop=True)
                nc.scalar.activation(
                    out=nxt[oc][:], in_=ps[:],
                    func=AF.Relu if i < 2 else AF.Identity,
                    bias=bpre[i][oc][:, 0:1], scale=1.0)
            cur = nxt
        F = cur  # [128, NJ] x2 chunks; cols 0:P_SHARD primary, rest compare

        # ---- stage 2: software-pipelined with a 3-deep skew ----
        # Emitting all stages of one group back-to-back serializes on the
        # xx -> l1 -> relu -> l2 -> relu -> l3 chain. Instead, iteration i
        # emits l3(i-3), l2(i-2), l1(i-1), xx(i): every engine's program
        # order then interleaves groups whose dependencies are already in
        # flight, so PE never waits on the same group's elementwise ops.
        NG = P_SHARD // PAIR
        if VARIANT == "half":
            NG = NG // 2
        xx_t, h0_t, h1_t, ps3_t = {}, {}, {}, {}

        def stage_xx(g):
            xx = [work.tile([128, NF], MM_DT, name=f"xx_{cc}", tag=f"xx_{cc}")
                  for cc in range(2)]
            for s in range(PAIR):
                p = g * PAIR + s
                for cc in range(2):
                    dst = xx[cc][:, s * CK:(s + 1) * CK]
                    if USE_GPSIMD_SUB:
                        d = work.tile([128, CK], F32, name=f"d_{cc}_{s}",
                                      tag=f"d_{cc}_{s}")
                        nc.gpsimd.tensor_scalar(
                            out=d[:], in0=F[cc][:, P_SHARD:NJ],
                            scalar1=F[cc][:, p:p + 1], scalar2=None,
                            op0=OP.subtract)
                        if (s + cc) % 2 == 0:
                            nc.scalar.activation(out=dst, in_=d[:],
                                                 func=AF.Square)
                        else:
                            nc.vector.tensor_tensor(out=dst, in0=d[:],
                                                    in1=d[:], op=OP.mult)
                    elif VARIANT == "xxD":
                        d = work.tile([128, CK], F32, name=f"d_{cc}_{s}",
                                      tag=f"d_{cc}_{s}")
                        nc.vector.tensor_scalar(
                            out=d[:], in0=F[cc][:, P_SHARD:NJ],
                            scalar1=F[cc][:, p:p + 1], scalar2=None,
                            op0=OP.subtract)
                        nc.vector.tensor_tensor(out=dst, in0=d[:],
                                                in1=d[:], op=OP.mult)
                    else:
                        nc.scalar.activation(
                            out=dst, in_=F[cc][:, P_SHARD:NJ],
                            func=AF.Square,
                            bias=F[cc][:, p:p + 1], scale=-1.0)
            xx_t[g] = xx

        def stage_mlp(g, l, hcur, out_map):
            odt = MM_DT if l == 0 else BF16
            hnxt = [work.tile([128, NF], odt, name=f"h{l}_{oc}",
                              tag=f"h{l}_{oc}") for oc in range(2)]
            for oc in range(2):
                ps = psum.tile([128, NF], F32, name=f"ps_l{l}_{oc}",
                               tag="ps_a" if l == 0 else "ps_b", bufs=3)
                nc.tensor.matmul(
                    out=ps[:],
                    lhsT=wpost[l][0][:, oc * 128:(oc + 1) * 128],
                    rhs=hcur[0][:], start=True, stop=False)
                nc.tensor.matmul(
                    out=ps[:],
                    lhsT=wpost[l][1][:, oc * 128:(oc + 1) * 128],
                    rhs=hcur[1][:], start=False, stop=True)
                on_act = (l + oc) % 2 == 0
                if VARIANT in ("reluA", "xxD"):
                    on_act = True
                elif VARIANT == "reluD":
                    on_act = False
                if on_act:
                    nc.scalar.activation(
                        out=hnxt[oc][:], in_=ps[:], func=AF.Relu,
                        bias=bpost[l][oc][:, 0:1], scale=1.0)
                else:
                    nc.vector.tensor_scalar(
                        out=hnxt[oc][:], in0=ps[:],
                        scalar1=bpost[l][oc][:, 0:1], scalar2=0.0,
                        op0=OP.add, op1=OP.max)
            out_map[g] = hnxt

        def stage_l3(g):
            # layer 3 (c->2): quad-packed into one PSUM bank -- group g%QUAD
            # computes at array col-group k, writing PSUM partitions
            # 32k..32k+1. Bias pre-seeded by a K=1 matmul that defines every
            # row. One DVE copy + 4 DMAs drain the quad.
            hcur = h1_t.pop(g)
            k = g % QUAD
            if k == 0:
                ps3 = psum.tile([128, NF], F32, name="ps3", tag="ps3", bufs=2)
                nc.tensor.matmul(out=ps3[:], lhsT=b3row[:], rhs=ones_row[:],
                                 start=True, stop=True)
                ps3_t[g // QUAD] = ps3
            ps3 = ps3_t[g // QUAD]
            nc.tensor.matmul(out=ps3[32 * k:32 * k + 2, :], lhsT=w3[0][:],
                             rhs=hcur[0][:], tile_position=(0, 32 * k),
                             start=False, stop=True, skip_group_check=True)
            nc.tensor.matmul(out=ps3[32 * k:32 * k + 2, :], lhsT=w3[1][:],
                             rhs=hcur[1][:], tile_position=(0, 32 * k),
                             start=False, stop=True, skip_group_check=True)
            if k == QUAD - 1:
                ps3_t.pop(g // QUAD)
                ob = work.tile([128, NF], F32, name="ob", tag="ob", bufs=2)
                nc.vector.tensor_copy(out=ob[:], in_=ps3[:])
                qb = (g // QUAD) * QUAD * PAIR
                for kk in range(QUAD):
                    nc.sync.dma_start(
                        out=out[:, qb + kk * PAIR:qb + (kk + 1) * PAIR, :],
                        in_=ob[32 * kk:32 * kk + 2, :].rearrange(
                            "j (s q) -> j s q", s=PAIR))

        for i in range(NG + 3):
            if i >= 3:
                stage_l3(i - 3)
            if 2 <= i < NG + 2:
                stage_mlp(i - 2, 1, h0_t.pop(i - 2), h1_t)
            if 1 <= i < NG + 1:
                stage_mlp(i - 1, 0, xx_t.pop(i - 1), h0_t)
            if i < NG:
                stage_xx(i)


_NC_CACHE = {}


def _get_nc():
    if "nc" not in _NC_CACHE:
        _NC_CACHE["nc"] = _build_nc()
    return _NC_CACHE["nc"]


def _shard_inputs(x, primary_indices, compare_indices, pre_w, pre_b,
                  post_w, post_b, post_out_w, post_out_b):
    """Host-side sharding: per-core index slicing + row gather of x (pure
    data movement -- the pre-MLP commutes with the gather) and weight
    transposes. Returns the 8 per-core input maps."""
    x = np.asarray(x, dtype=np.float32)
    x_rows = np.ascontiguousarray(x.transpose(0, 2, 3, 1)).reshape(B * H * W, C)
    pre_wT = np.ascontiguousarray(
        np.asarray(pre_w, dtype=np.float32).transpose(0, 2, 1))
    post_wT = np.ascontiguousarray(
        np.asarray(post_w, dtype=np.float32).transpose(0, 2, 1))
    import ml_dtypes
    w3T = np.ascontiguousarray(
        np.asarray(post_out_w, dtype=np.float32).T).astype(ml_dtypes.bfloat16)
    pre_b = np.ascontiguousarray(np.asarray(pre_b, dtype=np.float32))
    post_b = np.ascontiguousarray(np.asarray(post_b, dtype=np.float32))
    b3 = np.asarray(post_out_b, dtype=np.float32)
    b3row = np.zeros((1, 128), dtype=np.float32)
    for k in range(QUAD):
        b3row[0, 32 * k:32 * k + 2] = b3
    primary_indices = np.asarray(primary_indices)
    compare_indices = np.asarray(compare_indices)

    in_maps = []
    for core in range(N_CORES):
        b = core // CORES_PER_BATCH
        ps = (core % CORES_PER_BATCH) * P_SHARD
        rows = np.concatenate([
            primary_indices[b, ps:ps + P_SHARD].astype(np.int64),
            compare_indices[b].astype(np.int64),
        ])
        xg_T = np.ascontiguousarray(x_rows[rows].T)  # [C, NJ]
        in_maps.append({
            "xgT": xg_T,
            "pre_wT": pre_wT,
            "pre_b": pre_b,
            "post_wT": post_wT,
            "post_b": post_b,
            "w3T": w3T,
            "b3row": b3row,
            "ones_row": np.ones((1, NF), dtype=np.float32),
        })
    return in_maps


def _unshard_output(results):
    out = np.empty((B, 2, PK, CK), dtype=np.float32)
    for core in range(N_CORES):
        b = core // CORES_PER_BATCH
        ps = (core % CORES_PER_BATCH) * P_SHARD
        out[b, :, ps:ps + P_SHARD, :] = results[core]["out"]
    return out


def kernel(x, primary_indices, compare_indices, pre_w, pre_b,
           post_w, post_b, post_out_w, post_out_b):
    in_maps = _shard_inputs(x, primary_indices, compare_indices, pre_w, pre_b,
                            post_w, post_b, post_out_w, post_out_b)
    nc = _get_nc()
    res = run_bass_kernel_spmd(nc, in_maps, core_ids=list(range(N_CORES)))
    return _unshard_output(res.results)
